# revision 6
# baseline (speedup 1.0000x reference)
"""Multi-head self-attention Trainium2 kernel (8 NeuronCores, head-parallel).

Problem: L=4096, F_IN=1024, H=16, DH=64, F_OUT=1024, fp32.
Sharding: 2 heads per core (tensor parallel over heads). Each core computes
its 2 heads' attention and its partial output projection; the host sums the
8 partials (the all-reduce of the sharding hint, done at gather time).

Numerics: x and Wq/Wk/Wv are loaded in bf16 (phase 1 is DMA-bound on the
xT transfer; bf16 halves it to 8MB with negligible extra error since q/k/v
are stored bf16 anyway); projections accumulate in fp32 PSUM. The
attention/output matmuls run in bf16 (cheap LDWEIGHTS via fast-weight-load)
with fp32 PSUM accumulation. The softmax denominator is summed from the
*rounded* attention weights (ones-column trick), so bf16 rounding largely
cancels in the normalization; measured output error vs the fp32 reference
is ~4.5e-3 relative to the output absmax, HW exec ~385us.

Per-core pipeline:
  1. qT,kT,vT [128,4096] = W.T @ x.T from pre-transposed bf16 x (host
     side), processed in 1024-col quarters with double-buffered x tiles;
     v is turned into natural [j, d] layout via PE transpose-mode.
  2. Per i-chunk (512), per j-tile (128): scoresT for both heads as
     row-packed K=64 matmuls (concurrent sub-array execution) -> one ACT
     exp over the [128,1024] psum pair (ScalarE is the phase-2 roofline:
     ~1.11us per j-tile) -> attn@v with a ones-column on the stationary
     operand so PSUM row 64 accumulates the softmax denominators.
  3. Normalize + output projection are interleaved into the *next*
     i-chunk's j-loop to hide under the exp roof: evacuate vals (bf16) and
     denominators (fp32), reciprocal_approx_fast, gpsimd partition
     broadcast, normalize, out-proj (both heads accumulate into one PSUM
     bank), DMA out.

Bias handling: bq/bk folded into the ACT bias at qT/kT evacuation; bv is
exact as a host-side constant (softmax rows sum to 1 => out += sum_h
bv_h @ Wo_h); bo added on host. Phase-1/2 overlap relies on Tile's
range-granular dependency tracking (scores for j-tiles of quarter q start
as soon as that quarter's kT/vx are written).
"""

import numpy as np

L, F_IN, H, DH, F_OUT = 4096, 1024, 16, 64, 1024

# Schraudolph exp constants (DVE): int16(ps*SCH_A + SCH_B) bitcast bf16
SCH_C = 0.0579
SCH_A = 128.0 * 1.4426950408889634 * 0.125
SCH_B = 128.0 * (127.0 - SCH_C)
NCORES = 8
HPC = H // NCORES  # heads per core = 2
D2 = HPC * DH      # 128, per-core packed head dim

_BUILT = None


def _build():
    import os

    import concourse.bass as bass  # noqa: F401
    import concourse.mybir as mybir
    import concourse.tile as tile
    from concourse import bacc
    from concourse.masks import make_identity

    F = mybir.dt.float32
    FR = mybir.dt.float32r
    BF = mybir.dt.bfloat16
    I16 = mybir.dt.int16
    Alu = mybir.AluOpType
    Act = mybir.ActivationFunctionType

    nc = bacc.Bacc("TRN2", target_bir_lowering=False, debug=False)

    xT_d = nc.declare_dram_parameter("xT", [F_IN, L], BF, isOutput=False)
    wq_d = nc.declare_dram_parameter("wq", [F_IN, D2], BF, isOutput=False)
    wk_d = nc.declare_dram_parameter("wk", [F_IN, D2], BF, isOutput=False)
    wv_d = nc.declare_dram_parameter("wv", [F_IN, D2], BF, isOutput=False)
    bq_d = nc.declare_dram_parameter("bq", [D2], F, isOutput=False)
    bk_d = nc.declare_dram_parameter("bk", [D2], F, isOutput=False)
    wo0_d = nc.declare_dram_parameter("wo0", [DH, F_OUT], F, isOutput=False)
    wo1_d = nc.declare_dram_parameter("wo1", [DH, F_OUT], F, isOutput=False)
    out_d = nc.declare_dram_parameter("out", [L, F_OUT], F, isOutput=True)

    dbg = bool(os.environ.get("K_DEBUG"))
    if dbg:
        dbg_q = nc.declare_dram_parameter("dbg_q", [128, L], F, isOutput=True)
        dbg_k = nc.declare_dram_parameter("dbg_k", [128, L], F, isOutput=True)
        dbg_v = nc.declare_dram_parameter("dbg_v", [128, 32 * 65], F, isOutput=True)

    KT = F_IN // 128   # 8 f-tiles
    NI = L // 512      # 8 i-chunks
    NJ = L // 128      # 32 j-tiles
    QL = 1024          # quarter width in L
    NQ = L // QL       # 4 quarters

    with tile.TileContext(nc) as tc:
        with tc.tile_pool(name="persist", bufs=1) as pp:
            qT = pp.tile([128, L], BF, tag="qT")             # [d2, i]
            kT = pp.tile([128, L], BF, tag="kT")             # [d2, j]
            vx0 = pp.tile([128, NJ, DH + 1], BF, tag="vx0")  # [j_in, jt, d|1]
            vx1 = pp.tile([128, NJ, DH + 1], BF, tag="vx1")
            bq = pp.tile([128, 1], F, tag="bq")
            bk = pp.tile([128, 1], F, tag="bk")
            ones32 = pp.tile([128, NJ], F, tag="ones32")
            warm = pp.tile([1, 1], F, tag="warm")

            # pre-warm the exp table set while DMAs run
            nc.vector.memset(warm[:], 0.0)
            nc.scalar.activation(warm[:], warm[:], Act.Exp, scale=1.0)

            nc.vector.memset(ones32[:], 1.0)
            nc.vector.tensor_copy(vx0[:, :, DH:DH + 1], ones32[:, :, None])
            nc.vector.tensor_copy(vx1[:, :, DH:DH + 1], ones32[:, :, None])

            # Pools for the attention phase are opened before phase 1 is
            # emitted so the scheduler can overlap the phase-1 tail with
            # early score matmuls (PSUM: ps2s 4 + ps2v 2 + phase1 2 = 8).
            with tc.tile_pool(name="p2", bufs=1) as p2, \
                 tc.tile_pool(name="p2v", bufs=2) as p2v, \
                 tc.tile_pool(name="expp", bufs=12) as pe, \
                 tc.tile_pool(name="outp", bufs=4) as po, \
                 tc.tile_pool(name="ps2s", bufs=2, space="PSUM") as ps2s, \
                 tc.tile_pool(name="ps2v", bufs=1, space="PSUM") as ps2v:
                wo0 = p2.tile([DH, F_OUT], FR, tag="wo0")
                wo1 = p2.tile([DH, F_OUT], FR, tag="wo1")
                nc.sync.dma_start(out=wo0[:], in_=wo0_d.ap().bitcast(FR))
                nc.sync.dma_start(out=wo1[:], in_=wo1_d.ap().bitcast(FR))

                # ---- Phase 1: QKV projections over 4 quarters of L ----
                with tc.tile_pool(name="p1w", bufs=1) as p1w, \
                     tc.tile_pool(name="p1x", bufs=2) as p1x, \
                     tc.tile_pool(name="ps1", bufs=2, space="PSUM") as ps1:
                    wq = p1w.tile([128, KT, D2], BF, tag="wq")
                    wk = p1w.tile([128, KT, D2], BF, tag="wk")
                    wv = p1w.tile([128, KT, D2], BF, tag="wv")
                    ident = p1w.tile([128, 128], F, tag="ident")
                    for wt, wd in ((wk, wk_d), (wv, wv_d), (wq, wq_d)):
                        nc.sync.dma_start(
                            out=wt[:],
                            in_=wd.ap().rearrange("(k p) d -> p k d", p=128),
                        )
                    make_identity(nc, ident[:])
                    nc.sync.dma_start(out=bq[:], in_=bq_d.ap()[:, None])
                    nc.sync.dma_start(out=bk[:], in_=bk_d.ap()[:, None])

                    def proj(wt, dst, bias, xt, c0, g0):
                        ps = ps1.tile([128, 512], F, tag="ps1")
                        for kt in range(KT):
                            nc.tensor.matmul(
                                ps[:], wt[:, kt, :], xt[:, kt, c0:c0 + 512],
                                start=(kt == 0), stop=(kt == KT - 1),
                            )
                        if bias is not None:
                            nc.scalar.activation(
                                dst[:, g0:g0 + 512], ps[:], Act.Identity,
                                bias=bias[:], scale=1.0,
                            )
                        else:
                            nc.scalar.copy(dst[:, c0:c0 + 512], ps[:])

                    q_tiles = {}

                    def emit_q_dma(qq):
                        l0 = qq * QL
                        xt = p1x.tile([128, KT, QL], BF, tag="xt")
                        for kt in range(KT):
                            for hf in range(2):
                                h0 = hf * (QL // 2)
                                nc.sync.dma_start(
                                    out=xt[:, kt, h0:h0 + QL // 2],
                                    in_=xT_d.ap()[kt * 128:(kt + 1) * 128,
                                                  l0 + h0:l0 + h0 + QL // 2],
                                )
                        q_tiles[qq] = xt

                    def emit_q_comp(qq):
                        l0 = qq * QL
                        xt = q_tiles.pop(qq)
                        vTq = p1x.tile([128, QL], F, tag="vTq")
                        for ch in range(QL // 512):
                            proj(wk, kT, bk, xt, ch * 512, l0 + ch * 512)
                            proj(wv, vTq, None, xt, ch * 512, ch * 512)
                            if qq == 0:
                                proj(wq, qT, bq, xt, ch * 512, l0 + ch * 512)
                        for jl in range(QL // 128):
                            jt = qq * (QL // 128) + jl
                            pt = ps1.tile([128, 512], F, tag="ps1")
                            nc.tensor.transpose(
                                pt[:, 0:128],
                                vTq[:, jl * 128:(jl + 1) * 128], ident[:])
                            nc.vector.tensor_copy(vx0[:, jt, 0:DH], pt[:, 0:DH])
                            nc.vector.tensor_copy(vx1[:, jt, 0:DH],
                                                  pt[:, DH:D2])
                        if qq != 0:
                            for ch in range(QL // 512):
                                proj(wq, qT, bq, xt, ch * 512, l0 + ch * 512)

                    # chunk 0's attention interleaves into the remaining
                    # quarters so the in-order PE queue no longer serializes
                    # all of phase 1 ahead of the first scores matmul
                    emit_q_dma(0)
                    emit_q_comp(0)
                    emit_q_dma(1)
                    c0 = _phase2_chunk0(nc, ps2s, ps2v, pe, p2v, qT, kT,
                                        vx0, vx1, NJ, F, FR, BF, I16, Act,
                                        Alu, emit_q_dma, emit_q_comp)

                if dbg:
                    nc.sync.dma_start(out=dbg_q.ap(), in_=qT[:].bitcast(F))
                    nc.sync.dma_start(out=dbg_k.ap(), in_=kT[:].bitcast(F))
                    nc.sync.dma_start(
                        out=dbg_v.ap(),
                        in_=vx0[:].bitcast(F).rearrange("p a b -> p (a b)"))

                # ---- Phase 2+3: attention, interleaved normalize/out-proj ----
                with tc.tile_pool(name="ps2o", bufs=2, space="PSUM") as ps2o:
                    _phase2(nc, ps2s, ps2v, ps2o, pe, po, p2v,
                            qT, kT, vx0, vx1, wo0, wo1, out_d,
                            NI, NJ, F, FR, BF, I16, Act, Alu, c0)

    nc.compile()
    return nc


def _phase2(nc, ps2s, ps2v, ps2o, pe, po, p2v, qT, kT, vx0, vx1, wo0, wo1,
            out_d, NI, NJ, F, FR, BF, I16, Act, Alu, c0):
    def norm_unit(ic, p0, p1):
        # reciprocal + broadcast of the softmax denominators for chunk ic
        for (va, _), tg in ((p0, "0"), (p1, "1")):
            sh = p2v.tile([1, 512], F, tag="sh" + tg)
            rc = p2v.tile([1, 512], F, tag="rc" + tg)
            rb = p2v.tile([DH, 512], F, tag="rb" + tg)
            nc.sync.dma_start(out=sh[:], in_=va[DH:DH + 1, :].bitcast(F))
            nc.vector.reciprocal_approx_fast(out=rc[:], in_=sh[:])
            nc.gpsimd.partition_broadcast(rb[:], rc[:], channels=DH)
            nc.vector.tensor_mul(va[0:DH, :], va[0:DH, :], rb[:])

    def oproj_unit(ic, p0, p1, iw, fc, evac="s"):
        # one output-projection tile of chunk ic
        isl = slice(iw * 128, (iw + 1) * 128)
        r0 = ic * 512 + iw * 128
        f0 = fc * 512
        pso = ps2o.tile([128, 512], F, tag="pso")
        nc.tensor.matmul(
            pso[:], p0[0][0:DH, isl], wo0[:, f0:f0 + 512],
            start=True, stop=False,
        )
        nc.tensor.matmul(
            pso[:], p1[0][0:DH, isl], wo1[:, f0:f0 + 512],
            start=False, stop=True,
        )
        ot = po.tile([128, 512], F, tag="ot")
        if evac == "s":
            nc.scalar.copy(ot[:], pso[:])
        else:
            nc.vector.tensor_copy(ot[:], pso[:])
        nc.sync.dma_start(
            out=out_d.ap()[r0:r0 + 128, f0:f0 + 512], in_=ot[:])

    def emit_scores(ic, jt):
        i0 = ic * 512
        j0 = jt * 128
        ps = ps2s.tile([128, 1024], F, tag="pss")
        nc.tensor.matmul(
            ps[:, 0:512], kT[0:64, j0:j0 + 128], qT[0:64, i0:i0 + 512],
            start=True, stop=True, tile_position=(0, 0),
        )
        nc.tensor.matmul(
            ps[:, 512:1024], kT[64:128, j0:j0 + 128],
            qT[64:128, i0:i0 + 512],
            start=True, stop=True, tile_position=(64, 0),
        )
        return ps

    # One-step score skew: scores for step n+1 are emitted between exp(n)
    # and vals(n), so on TensorE's FIFO they are not blocked behind the
    # exp(n)-gated vals, and exp(n+1)'s input is ready a full step early.
    def emit_attnv_block(pv0, pv1, eTs, b0):
        for bjt in range(b0, b0 + 8):
            nc.tensor.matmul(
                pv0[:], vx0[:, bjt, :], eTs[bjt][:, 0:512],
                start=(bjt == 0), stop=(bjt == NJ - 1),
            )
        for bjt in range(b0, b0 + 8):
            nc.tensor.matmul(
                pv1[:], vx1[:, bjt, :], eTs[bjt][:, 512:1024],
                start=(bjt == 0), stop=(bjt == NJ - 1),
            )

    pending = c0["pending"]
    ps_next = c0["ps_next"]
    for ic in range(1, NI):
        pv0 = ps2v.tile([DH + 1, 512], F, tag="pv0")
        pv1 = ps2v.tile([DH + 1, 512], F, tag="pv1")
        # out-proj units of the previous chunk, spread through this jt loop
        units = []
        if pending is not None:
            pic, pp0, pp1 = pending
            # evac engine per unit: ScalarE takes 6 of 8 (DVE carries half
            # the exps now, ScalarE has the spare slots)
            units = [(pic, pp0, pp1, iw, fc,
                      "v" if (iw * 2 + fc) % 4 == 3 else "s")
                     for iw in range(4) for fc in range(F_OUT // 512)]
        eTs = []
        for jt in range(NJ):
            ps = ps_next
            eT = pe.tile([128, 1024], BF, tag="eT")
            # Strict ACT/DVE alternation: the scores PSUM bank of tile n is
            # freed by exp(n), which gates scores(n+2) (2 buffers).  With
            # exps ping-ponging between the two engines the bank-free
            # cadence is (213+sem+~1180)/2 per tile instead of the serial
            # ScalarE rate that paced the old 3-of-4 split.
            if jt % 2 == 1:
                nc.vector.tensor_scalar(
                    eT[:].bitcast(I16), ps[:], SCH_A, SCH_B,
                    Alu.mult, Alu.add)
            else:
                nc.scalar.activation(eT[:], ps[:], Act.Exp, scale=0.125)
            eTs.append(eT)
            n = ic * NJ + jt
            if n + 1 < NI * NJ:
                ps_next = emit_scores((n + 1) // NJ, (n + 1) % NJ)
            # attn@v is emitted in 4-tile chain blocks lagged one tile
            # (newest dep = exp(jt-1), already done when PE arrives): the
            # per-head accumulation matmuls run back-to-back so only the
            # first of each block pays the PE group-switch cost, and the
            # PE never stalls on a still-running exp before reaching the
            # next scores pass
            if jt % 8 == 0 and jt > 0:
                emit_attnv_block(pv0, pv1, eTs, jt - 8)
            if jt == 0 and pending is not None:
                norm_unit(*pending)
            if jt % 4 == 1 and 5 <= jt < 31 and units:
                oproj_unit(*units.pop(0))
        emit_attnv_block(pv0, pv1, eTs, NJ - 8)
        for u in units:
            oproj_unit(*u)

        # evacuate this chunk's vals+denominators in one fp32r copy per
        # head: the next chunk's first vals matmul reclaims the PSUM bank
        # after ~0.7us instead of waiting a 4-copy chain
        va0 = p2v.tile([DH + 1, 512], FR, tag="va0")
        va1 = p2v.tile([DH + 1, 512], FR, tag="va1")
        nc.vector.tensor_copy(va0[:], pv0[:])
        nc.vector.tensor_copy(va1[:], pv1[:])
        pending = (ic, (va0, None), (va1, None))

    norm_unit(*pending)
    pic, pp0, pp1 = pending
    for iw in range(4):
        for fc in range(F_OUT // 512):
            oproj_unit(pic, pp0, pp1, iw, fc, "s" if fc == 0 else "v")


def _get_built():
    global _BUILT
    if _BUILT is None:
        _BUILT = _build()
    return _BUILT


def kernel(x, Wq, bq, Wk, bk, Wv, bv, Wo, bo):
    from concourse.bass_utils import run_bass_kernel_spmd

    x = np.ascontiguousarray(np.asarray(x, dtype=np.float32))
    Wq = np.asarray(Wq, dtype=np.float32)
    Wk = np.asarray(Wk, dtype=np.float32)
    Wv = np.asarray(Wv, dtype=np.float32)
    Wo = np.asarray(Wo, dtype=np.float32)
    bq = np.asarray(bq, dtype=np.float32)
    bk = np.asarray(bk, dtype=np.float32)
    bv = np.asarray(bv, dtype=np.float32)
    bo = np.asarray(bo, dtype=np.float32)

    nc = _get_built()

    import ml_dtypes
    BFH = ml_dtypes.bfloat16
    xT = np.ascontiguousarray(x.T.astype(BFH))  # [F_IN, L] bf16
    in_maps = []
    for c in range(NCORES):
        hs = slice(c * HPC, (c + 1) * HPC)
        in_maps.append({
            "xT": xT,
            "wq": np.ascontiguousarray(
                Wq[:, hs, :].reshape(F_IN, D2).astype(BFH)),
            "wk": np.ascontiguousarray(
                Wk[:, hs, :].reshape(F_IN, D2).astype(BFH)),
            "wv": np.ascontiguousarray(
                Wv[:, hs, :].reshape(F_IN, D2).astype(BFH)),
            "bq": np.ascontiguousarray(bq[hs].reshape(D2)),
            "bk": np.ascontiguousarray(bk[hs].reshape(D2)),
            "wo0": np.ascontiguousarray(Wo[c * HPC]),
            "wo1": np.ascontiguousarray(Wo[c * HPC + 1]),
        })

    res = run_bass_kernel_spmd(nc, in_maps, list(range(NCORES)))
    acc = np.zeros((L, F_OUT), dtype=np.float64)
    for c in range(NCORES):
        acc += res.results[c]["out"].astype(np.float64)
    # bv contribution (softmax rows sum to 1) + bo, both exact on host
    acc += (bv.reshape(1, H * DH).astype(np.float64)
            @ Wo.reshape(H * DH, F_OUT).astype(np.float64))
    acc += bo.astype(np.float64)
    return acc.astype(np.float32)


def _phase2_chunk0(nc, ps2s, ps2v, pe, p2v, qT, kT, vx0, vx1, NJ, F, FR,
                   BF, I16, Act, Alu, emit_q_dma, emit_q_comp):
    """Chunk 0's j-loop, emitted inside the phase-1 pool scope with the
    remaining quarters' DMAs/compute interleaved at fixed j-slots."""
    def emit_scores(ic, jt):
        i0 = ic * 512
        j0 = jt * 128
        ps = ps2s.tile([128, 1024], F, tag="pss")
        nc.tensor.matmul(
            ps[:, 0:512], kT[0:64, j0:j0 + 128], qT[0:64, i0:i0 + 512],
            start=True, stop=True, tile_position=(0, 0),
        )
        nc.tensor.matmul(
            ps[:, 512:1024], kT[64:128, j0:j0 + 128],
            qT[64:128, i0:i0 + 512],
            start=True, stop=True, tile_position=(64, 0),
        )
        return ps

    pv0 = ps2v.tile([DH + 1, 512], F, tag="pv0")
    pv1 = ps2v.tile([DH + 1, 512], F, tag="pv1")
    eTs = []
    ps_next = emit_scores(0, 0)
    for jt in range(NJ):
        ps = ps_next
        eT = pe.tile([128, 1024], BF, tag="eT")
        if jt % 2 == 1:
            nc.vector.tensor_scalar(
                eT[:].bitcast(I16), ps[:], SCH_A, SCH_B, Alu.mult, Alu.add)
        else:
            nc.scalar.activation(eT[:], ps[:], Act.Exp, scale=0.125)
        eTs.append(eT)
        if jt % 8 == 6 and jt < NJ - 2:
            emit_q_comp(jt // 8 + 1)
            if jt // 8 + 2 < 4:
                emit_q_dma(jt // 8 + 2)
        ps_next = emit_scores((jt + 1) // NJ, (jt + 1) % NJ)
        if jt % 8 == 0 and jt > 0:
            for bjt in range(jt - 8, jt):
                nc.tensor.matmul(
                    pv0[:], vx0[:, bjt, :], eTs[bjt][:, 0:512],
                    start=(bjt == 0), stop=(bjt == NJ - 1),
                )
            for bjt in range(jt - 8, jt):
                nc.tensor.matmul(
                    pv1[:], vx1[:, bjt, :], eTs[bjt][:, 512:1024],
                    start=(bjt == 0), stop=(bjt == NJ - 1),
                )
    for bjt in range(NJ - 8, NJ):
        nc.tensor.matmul(
            pv0[:], vx0[:, bjt, :], eTs[bjt][:, 0:512],
            start=(bjt == 0), stop=(bjt == NJ - 1),
        )
    for bjt in range(NJ - 8, NJ):
        nc.tensor.matmul(
            pv1[:], vx1[:, bjt, :], eTs[bjt][:, 512:1024],
            start=(bjt == 0), stop=(bjt == NJ - 1),
        )
    va0 = p2v.tile([DH + 1, 512], FR, tag="va0")
    va1 = p2v.tile([DH + 1, 512], FR, tag="va1")
    nc.vector.tensor_copy(va0[:], pv0[:])
    nc.vector.tensor_copy(va1[:], pv1[:])
    return {"pending": (0, (va0, None), (va1, None)), "ps_next": ps_next}



# revision 11
# speedup vs baseline: 1.0185x; 1.0185x over previous
"""Multi-head self-attention Trainium2 kernel (8 NeuronCores, head-parallel).

Problem: L=4096, F_IN=1024, H=16, DH=64, F_OUT=1024, fp32.
Sharding: 2 heads per core (tensor parallel over heads). Each core computes
its 2 heads' attention and its partial output projection; the host sums the
8 partials (the all-reduce of the sharding hint, done at gather time).

Numerics: x and Wq/Wk/Wv are loaded in bf16 (phase 1 is DMA-bound on the
xT transfer; bf16 halves it to 8MB with negligible extra error since q/k/v
are stored bf16 anyway); projections accumulate in fp32 PSUM. The
attention/output matmuls run in bf16 (cheap LDWEIGHTS via fast-weight-load)
with fp32 PSUM accumulation. The softmax denominator is summed from the
*rounded* attention weights (ones-column trick), so bf16 rounding largely
cancels in the normalization; measured output error vs the fp32 reference
is ~4.5e-3 relative to the output absmax, HW exec ~385us.

Per-core pipeline:
  1. qT,kT,vT [128,4096] = W.T @ x.T from pre-transposed bf16 x (host
     side), processed in 1024-col quarters with double-buffered x tiles;
     v is turned into natural [j, d] layout via PE transpose-mode.
  2. Per i-chunk (512), per j-tile (128): scoresT for both heads as
     row-packed K=64 matmuls (concurrent sub-array execution) -> one ACT
     exp over the [128,1024] psum pair (ScalarE is the phase-2 roofline:
     ~1.11us per j-tile) -> attn@v with a ones-column on the stationary
     operand so PSUM row 64 accumulates the softmax denominators.
  3. Normalize + output projection are interleaved into the *next*
     i-chunk's j-loop to hide under the exp roof: evacuate vals (bf16) and
     denominators (fp32), reciprocal_approx_fast, gpsimd partition
     broadcast, normalize, out-proj (both heads accumulate into one PSUM
     bank), DMA out.

Bias handling: bq/bk folded into the ACT bias at qT/kT evacuation; bv is
exact as a host-side constant (softmax rows sum to 1 => out += sum_h
bv_h @ Wo_h); bo added on host. Phase-1/2 overlap relies on Tile's
range-granular dependency tracking (scores for j-tiles of quarter q start
as soon as that quarter's kT/vx are written).
"""

import numpy as np

L, F_IN, H, DH, F_OUT = 4096, 1024, 16, 64, 1024

# Schraudolph exp constants (DVE): int16(ps*SCH_A + SCH_B) bitcast bf16
SCH_C = 0.0579
SCH_A = 128.0 * 1.4426950408889634 * 0.125
SCH_B = 128.0 * (127.0 - SCH_C)
NCORES = 8
HPC = H // NCORES  # heads per core = 2
D2 = HPC * DH      # 128, per-core packed head dim

_BUILT = None


def _build():
    import os

    import concourse.bass as bass  # noqa: F401
    import concourse.mybir as mybir
    import concourse.tile as tile
    from concourse import bacc
    from concourse.masks import make_identity

    F = mybir.dt.float32
    FR = mybir.dt.float32r
    BF = mybir.dt.bfloat16
    I16 = mybir.dt.int16
    Alu = mybir.AluOpType
    Act = mybir.ActivationFunctionType

    nc = bacc.Bacc("TRN2", target_bir_lowering=False, debug=False)

    xT_d = nc.declare_dram_parameter("xT", [F_IN, L], BF, isOutput=False)
    wq_d = nc.declare_dram_parameter("wq", [F_IN, D2], BF, isOutput=False)
    wk_d = nc.declare_dram_parameter("wk", [F_IN, D2], BF, isOutput=False)
    wv_d = nc.declare_dram_parameter("wv", [F_IN, D2], BF, isOutput=False)
    bq_d = nc.declare_dram_parameter("bq", [D2], F, isOutput=False)
    bk_d = nc.declare_dram_parameter("bk", [D2], F, isOutput=False)
    wo0_d = nc.declare_dram_parameter("wo0", [DH, F_OUT], F, isOutput=False)
    wo1_d = nc.declare_dram_parameter("wo1", [DH, F_OUT], F, isOutput=False)
    out_d = nc.declare_dram_parameter("out", [L, F_OUT], F, isOutput=True)

    dbg = bool(os.environ.get("K_DEBUG"))
    if dbg:
        dbg_q = nc.declare_dram_parameter("dbg_q", [128, L], F, isOutput=True)
        dbg_k = nc.declare_dram_parameter("dbg_k", [128, L], F, isOutput=True)
        dbg_v = nc.declare_dram_parameter("dbg_v", [128, 32 * 65], F, isOutput=True)

    KT = F_IN // 128   # 8 f-tiles
    NI = L // 512      # 8 i-chunks
    NJ = L // 128      # 32 j-tiles
    QL = 1024          # quarter width in L
    NQ = L // QL       # 4 quarters

    with tile.TileContext(nc) as tc:
        with tc.tile_pool(name="persist", bufs=1) as pp:
            qT = pp.tile([128, L], BF, tag="qT")             # [d2, i]
            kT = pp.tile([128, L], BF, tag="kT")             # [d2, j]
            vx0 = pp.tile([128, NJ, DH + 1], BF, tag="vx0")  # [j_in, jt, d|1]
            vx1 = pp.tile([128, NJ, DH + 1], BF, tag="vx1")
            bq = pp.tile([128, 1], F, tag="bq")
            bk = pp.tile([128, 1], F, tag="bk")
            ones32 = pp.tile([128, NJ], F, tag="ones32")
            warm = pp.tile([1, 1], F, tag="warm")

            # pre-warm the exp table set while DMAs run
            nc.vector.memset(warm[:], 0.0)
            nc.scalar.activation(warm[:], warm[:], Act.Exp, scale=1.0)

            nc.vector.memset(ones32[:], 1.0)
            nc.vector.tensor_copy(vx0[:, :, DH:DH + 1], ones32[:, :, None])
            nc.vector.tensor_copy(vx1[:, :, DH:DH + 1], ones32[:, :, None])

            # Pools for the attention phase are opened before phase 1 is
            # emitted so the scheduler can overlap the phase-1 tail with
            # early score matmuls (PSUM: ps2s 4 + ps2v 2 + phase1 2 = 8).
            with tc.tile_pool(name="p2", bufs=1) as p2, \
                 tc.tile_pool(name="p2v", bufs=2) as p2v, \
                 tc.tile_pool(name="expp", bufs=20) as pe, \
                 tc.tile_pool(name="outp", bufs=4) as po, \
                 tc.tile_pool(name="ps2s", bufs=2, space="PSUM") as ps2s, \
                 tc.tile_pool(name="ps2v", bufs=1, space="PSUM") as ps2v:
                wo0 = p2.tile([DH, F_OUT], FR, tag="wo0")
                wo1 = p2.tile([DH, F_OUT], FR, tag="wo1")
                nc.sync.dma_start(out=wo0[:], in_=wo0_d.ap().bitcast(FR))
                nc.sync.dma_start(out=wo1[:], in_=wo1_d.ap().bitcast(FR))

                # ---- Phase 1: QKV projections over 4 quarters of L ----
                with tc.tile_pool(name="p1w", bufs=1) as p1w, \
                     tc.tile_pool(name="p1x", bufs=2) as p1x, \
                     tc.tile_pool(name="ps1", bufs=2, space="PSUM") as ps1:
                    wq = p1w.tile([128, KT, D2], BF, tag="wq")
                    wk = p1w.tile([128, KT, D2], BF, tag="wk")
                    wv = p1w.tile([128, KT, D2], BF, tag="wv")
                    ident = p1w.tile([128, 128], F, tag="ident")
                    for wt, wd in ((wk, wk_d), (wv, wv_d), (wq, wq_d)):
                        nc.sync.dma_start(
                            out=wt[:],
                            in_=wd.ap().rearrange("(k p) d -> p k d", p=128),
                        )
                    make_identity(nc, ident[:])
                    nc.sync.dma_start(out=bq[:], in_=bq_d.ap()[:, None])
                    nc.sync.dma_start(out=bk[:], in_=bk_d.ap()[:, None])

                    def proj(wt, dst, bias, xt, c0, g0):
                        ps = ps1.tile([128, 512], F, tag="ps1")
                        for kt in range(KT):
                            nc.tensor.matmul(
                                ps[:], wt[:, kt, :], xt[:, kt, c0:c0 + 512],
                                start=(kt == 0), stop=(kt == KT - 1),
                            )
                        if bias is not None:
                            nc.scalar.activation(
                                dst[:, g0:g0 + 512], ps[:], Act.Identity,
                                bias=bias[:], scale=1.0,
                            )
                        else:
                            nc.scalar.copy(dst[:, c0:c0 + 512], ps[:])

                    q_tiles = {}

                    def emit_q_dma(qq):
                        l0 = qq * QL
                        xt = p1x.tile([128, KT, QL], BF, tag="xt")
                        for kt in range(KT):
                            for hf in range(2):
                                h0 = hf * (QL // 2)
                                nc.sync.dma_start(
                                    out=xt[:, kt, h0:h0 + QL // 2],
                                    in_=xT_d.ap()[kt * 128:(kt + 1) * 128,
                                                  l0 + h0:l0 + h0 + QL // 2],
                                )
                        q_tiles[qq] = xt

                    def emit_q_comp(qq):
                        l0 = qq * QL
                        xt = q_tiles.pop(qq)
                        vTq = p1x.tile([128, QL], F, tag="vTq")
                        for ch in range(QL // 512):
                            proj(wk, kT, bk, xt, ch * 512, l0 + ch * 512)
                            proj(wv, vTq, None, xt, ch * 512, ch * 512)
                            if qq == 0:
                                proj(wq, qT, bq, xt, ch * 512, l0 + ch * 512)
                        for jl in range(QL // 128):
                            jt = qq * (QL // 128) + jl
                            pt = ps1.tile([128, 512], F, tag="ps1")
                            nc.tensor.transpose(
                                pt[:, 0:128],
                                vTq[:, jl * 128:(jl + 1) * 128], ident[:])
                            nc.vector.tensor_copy(vx0[:, jt, 0:DH], pt[:, 0:DH])
                            nc.vector.tensor_copy(vx1[:, jt, 0:DH],
                                                  pt[:, DH:D2])
                        if qq != 0:
                            for ch in range(QL // 512):
                                proj(wq, qT, bq, xt, ch * 512, l0 + ch * 512)

                    # chunk 0's attention interleaves into the remaining
                    # quarters so the in-order PE queue no longer serializes
                    # all of phase 1 ahead of the first scores matmul
                    emit_q_dma(0)
                    emit_q_comp(0)
                    emit_q_dma(1)
                    c0 = _phase2_chunk0(nc, ps2s, ps2v, pe, p2v, qT, kT,
                                        vx0, vx1, NJ, F, FR, BF, I16, Act,
                                        Alu, emit_q_dma, emit_q_comp)

                if dbg:
                    nc.sync.dma_start(out=dbg_q.ap(), in_=qT[:].bitcast(F))
                    nc.sync.dma_start(out=dbg_k.ap(), in_=kT[:].bitcast(F))
                    nc.sync.dma_start(
                        out=dbg_v.ap(),
                        in_=vx0[:].bitcast(F).rearrange("p a b -> p (a b)"))

                # ---- Phase 2+3: attention, interleaved normalize/out-proj ----
                with tc.tile_pool(name="ps2o", bufs=1, space="PSUM") as ps2o:
                    _phase2(nc, ps2s, ps2v, ps2o, pe, po, p2v,
                            qT, kT, vx0, vx1, wo0, wo1, out_d,
                            NI, NJ, F, FR, BF, I16, Act, Alu, c0)

    nc.compile()
    return nc


def _phase2(nc, ps2s, ps2v, ps2o, pe, po, p2v, qT, kT, vx0, vx1, wo0, wo1,
            out_d, NI, NJ, F, FR, BF, I16, Act, Alu, c0):
    def norm_unit(ic, p0, p1):
        # reciprocal + broadcast of the softmax denominators for chunk ic
        for (va, _), tg in ((p0, "0"), (p1, "1")):
            sh = p2v.tile([1, 512], F, tag="sh" + tg)
            rc = p2v.tile([1, 512], F, tag="rc" + tg)
            rb = p2v.tile([DH, 512], F, tag="rb" + tg)
            nc.sync.dma_start(out=sh[:], in_=va[DH:DH + 1, :].bitcast(F))
            nc.vector.reciprocal_approx_fast(out=rc[:], in_=sh[:])
            nc.gpsimd.partition_broadcast(rb[:], rc[:], channels=DH)
            nc.vector.tensor_mul(va[0:DH, :], va[0:DH, :], rb[:])

    def oproj_unit(ic, p0, p1, iw, evac="s"):
        # one output-projection row-block of chunk ic: both 512-wide f
        # chunks into one 2-bank PSUM tile, single evac + single DMA
        isl = slice(iw * 128, (iw + 1) * 128)
        r0 = ic * 512 + iw * 128
        pso = ps2o.tile([128, 1024], F, tag="pso")
        for fc in range(2):
            f0 = fc * 512
            nc.tensor.matmul(
                pso[:, f0:f0 + 512], p0[0][0:DH, isl], wo0[:, f0:f0 + 512],
                start=True, stop=False,
            )
            nc.tensor.matmul(
                pso[:, f0:f0 + 512], p1[0][0:DH, isl], wo1[:, f0:f0 + 512],
                start=False, stop=True,
            )
        ot = po.tile([128, 1024], F, tag="ot")
        if evac == "s":
            nc.scalar.copy(ot[:], pso[:])
        else:
            nc.vector.tensor_copy(ot[:], pso[:])
        nc.sync.dma_start(out=out_d.ap()[r0:r0 + 128, :], in_=ot[:])

    def emit_scores(ic, jt):
        i0 = ic * 512
        j0 = jt * 128
        ps = ps2s.tile([128, 1024], F, tag="pss")
        nc.tensor.matmul(
            ps[:, 0:512], kT[0:64, j0:j0 + 128], qT[0:64, i0:i0 + 512],
            start=True, stop=True, tile_position=(0, 0),
        )
        nc.tensor.matmul(
            ps[:, 512:1024], kT[64:128, j0:j0 + 128],
            qT[64:128, i0:i0 + 512],
            start=True, stop=True, tile_position=(64, 0),
        )
        return ps

    def emit_attnv_block(pv0, pv1, eTs, b0):
        for bjt in range(b0, b0 + 8):
            nc.tensor.matmul(
                pv0[:], vx0[:, bjt, :], eTs[bjt][:, 0:512],
                start=(bjt == 0), stop=(bjt == NJ - 1),
            )
        for bjt in range(b0, b0 + 8):
            nc.tensor.matmul(
                pv1[:], vx1[:, bjt, :], eTs[bjt][:, 512:1024],
                start=(bjt == 0), stop=(bjt == NJ - 1),
            )

    # Emission order per jt: exp first, then all PE work whose deps are
    # long-satisfied (attn@v lagged 16 tiles, out-proj of the previous
    # chunk), and the bank-gated scores(n+1) LAST so its stall cannot
    # head-of-line-block ready matmuls, whose PSUM results feed the
    # ACT/DVE evacuation slots between exps.  Chunk-end work (final
    # attn@v block, pv evac, norm) is deferred into the next chunk's
    # first iterations for the same reason.
    pending = c0["pending"]
    ps_next = c0["ps_next"]
    carry_block = c0["carry_block"]
    for ic in range(1, NI):
        units = []
        if pending is not None:
            pic, pp0, pp1 = pending[0], pending[1], pending[2]
            units = [(pic, pp0, pp1, iw, "s" if iw % 2 == 0 else "v")
                     for iw in range(4)]
        eTs = []
        pv0 = pv1 = None
        for jt in range(NJ):
            ps = ps_next
            eT = pe.tile([128, 1024], BF, tag="eT")
            if jt % 2 == 1:
                nc.vector.tensor_scalar(
                    eT[:].bitcast(I16), ps[:], SCH_A, SCH_B,
                    Alu.mult, Alu.add)
            else:
                nc.scalar.activation(eT[:], ps[:], Act.Exp, scale=0.125)
            eTs.append(eT)
            if jt == 0 and carry_block is not None:
                carry_block()
                carry_block = None
            if jt == 1:
                # evacuate previous chunk's vals+denominators (fp32r)
                if pending is not None:
                    nc.vector.tensor_copy(pending[1][0][:], pending[3][:])
                    nc.vector.tensor_copy(pending[2][0][:], pending[4][:])
                pv0 = ps2v.tile([DH + 1, 512], F, tag="pv0")
                pv1 = ps2v.tile([DH + 1, 512], F, tag="pv1")
            if jt == 3 and pending is not None:
                norm_unit(pending[0], pending[1], pending[2])
            if jt % 8 == 0 and jt >= 16:
                emit_attnv_block(pv0, pv1, eTs, jt - 16)
            if jt % 6 == 5 and units:
                oproj_unit(*units.pop(0))
            n = ic * NJ + jt
            if n + 1 < NI * NJ:
                ps_next = emit_scores((n + 1) // NJ, (n + 1) % NJ)
        emit_attnv_block(pv0, pv1, eTs, NJ - 16)
        va0 = p2v.tile([DH + 1, 512], FR, tag="va0")
        va1 = p2v.tile([DH + 1, 512], FR, tag="va1")
        if ic < NI - 1:
            cpv0, cpv1, ceTs = pv0, pv1, eTs
            carry_block = (lambda p0=cpv0, p1=cpv1, e=ceTs:
                           emit_attnv_block(p0, p1, e, NJ - 8))
            pending = (ic, (va0, None), (va1, None), pv0, pv1)
        else:
            emit_attnv_block(pv0, pv1, eTs, NJ - 8)
            nc.vector.tensor_copy(va0[:], pv0[:])
            nc.vector.tensor_copy(va1[:], pv1[:])
            pending = (ic, (va0, None), (va1, None), pv0, pv1)

    norm_unit(pending[0], pending[1], pending[2])
    for iw in range(4):
        oproj_unit(pending[0], pending[1], pending[2], iw,
                   "s" if iw % 2 == 0 else "v")


def _get_built():
    global _BUILT
    if _BUILT is None:
        _BUILT = _build()
    return _BUILT


def kernel(x, Wq, bq, Wk, bk, Wv, bv, Wo, bo):
    from concourse.bass_utils import run_bass_kernel_spmd

    x = np.ascontiguousarray(np.asarray(x, dtype=np.float32))
    Wq = np.asarray(Wq, dtype=np.float32)
    Wk = np.asarray(Wk, dtype=np.float32)
    Wv = np.asarray(Wv, dtype=np.float32)
    Wo = np.asarray(Wo, dtype=np.float32)
    bq = np.asarray(bq, dtype=np.float32)
    bk = np.asarray(bk, dtype=np.float32)
    bv = np.asarray(bv, dtype=np.float32)
    bo = np.asarray(bo, dtype=np.float32)

    nc = _get_built()

    import ml_dtypes
    BFH = ml_dtypes.bfloat16
    xT = np.ascontiguousarray(x.T.astype(BFH))  # [F_IN, L] bf16
    in_maps = []
    for c in range(NCORES):
        hs = slice(c * HPC, (c + 1) * HPC)
        in_maps.append({
            "xT": xT,
            "wq": np.ascontiguousarray(
                Wq[:, hs, :].reshape(F_IN, D2).astype(BFH)),
            "wk": np.ascontiguousarray(
                Wk[:, hs, :].reshape(F_IN, D2).astype(BFH)),
            "wv": np.ascontiguousarray(
                Wv[:, hs, :].reshape(F_IN, D2).astype(BFH)),
            "bq": np.ascontiguousarray(bq[hs].reshape(D2)),
            "bk": np.ascontiguousarray(bk[hs].reshape(D2)),
            "wo0": np.ascontiguousarray(Wo[c * HPC]),
            "wo1": np.ascontiguousarray(Wo[c * HPC + 1]),
        })

    res = run_bass_kernel_spmd(nc, in_maps, list(range(NCORES)))
    acc = np.zeros((L, F_OUT), dtype=np.float64)
    for c in range(NCORES):
        acc += res.results[c]["out"].astype(np.float64)
    # bv contribution (softmax rows sum to 1) + bo, both exact on host
    acc += (bv.reshape(1, H * DH).astype(np.float64)
            @ Wo.reshape(H * DH, F_OUT).astype(np.float64))
    acc += bo.astype(np.float64)
    return acc.astype(np.float32)


def _phase2_chunk0(nc, ps2s, ps2v, pe, p2v, qT, kT, vx0, vx1, NJ, F, FR,
                   BF, I16, Act, Alu, emit_q_dma, emit_q_comp):
    """Chunk 0's j-loop, emitted inside the phase-1 pool scope with the
    remaining quarters' DMAs/compute interleaved at fixed j-slots."""
    def emit_scores(ic, jt):
        i0 = ic * 512
        j0 = jt * 128
        ps = ps2s.tile([128, 1024], F, tag="pss")
        nc.tensor.matmul(
            ps[:, 0:512], kT[0:64, j0:j0 + 128], qT[0:64, i0:i0 + 512],
            start=True, stop=True, tile_position=(0, 0),
        )
        nc.tensor.matmul(
            ps[:, 512:1024], kT[64:128, j0:j0 + 128],
            qT[64:128, i0:i0 + 512],
            start=True, stop=True, tile_position=(64, 0),
        )
        return ps

    def emit_attnv_block(pv0, pv1, eTs, b0):
        for bjt in range(b0, b0 + 8):
            nc.tensor.matmul(
                pv0[:], vx0[:, bjt, :], eTs[bjt][:, 0:512],
                start=(bjt == 0), stop=(bjt == NJ - 1),
            )
        for bjt in range(b0, b0 + 8):
            nc.tensor.matmul(
                pv1[:], vx1[:, bjt, :], eTs[bjt][:, 512:1024],
                start=(bjt == 0), stop=(bjt == NJ - 1),
            )

    pv0 = ps2v.tile([DH + 1, 512], F, tag="pv0")
    pv1 = ps2v.tile([DH + 1, 512], F, tag="pv1")
    eTs = []
    ps_next = emit_scores(0, 0)
    for jt in range(NJ):
        ps = ps_next
        eT = pe.tile([128, 1024], BF, tag="eT")
        if jt % 2 == 1:
            nc.vector.tensor_scalar(
                eT[:].bitcast(I16), ps[:], SCH_A, SCH_B, Alu.mult, Alu.add)
        else:
            nc.scalar.activation(eT[:], ps[:], Act.Exp, scale=0.125)
        eTs.append(eT)
        if jt % 8 == 0 and jt >= 16:
            emit_attnv_block(pv0, pv1, eTs, jt - 16)
        if jt % 8 == 6 and jt < NJ - 2:
            emit_q_comp(jt // 8 + 1)
            if jt // 8 + 2 < 4:
                emit_q_dma(jt // 8 + 2)
        ps_next = emit_scores((jt + 1) // NJ, (jt + 1) % NJ)
    emit_attnv_block(pv0, pv1, eTs, NJ - 16)
    va0 = p2v.tile([DH + 1, 512], FR, tag="va0")
    va1 = p2v.tile([DH + 1, 512], FR, tag="va1")
    carry = (lambda p0=pv0, p1=pv1, e=eTs: emit_attnv_block(p0, p1, e, NJ - 8))
    return {"pending": (0, (va0, None), (va1, None), pv0, pv1),
            "ps_next": ps_next, "carry_block": carry}

